# revision 4
# baseline (speedup 1.0000x reference)
"""Conv2d(128->256, 3x3, pad=1) over (32,128,56,56), data-parallel across 8
NeuronCores (4 images per core).

Per core: conv = 9 shifted accumulating matmuls per output tile.
  - contraction K = Cin = 128 (partition dim)
  - stationary lhsT = W^T[ci, co_tile] per (ky,kx)  -> [128, 128] bf16
  - moving rhs = input pixels [128, <=8 rows, <=56 cols] (N <= 448)
  - PSUM accumulates the 9 (ky,kx) taps; padding handled by clipping each
    tap's matmul to the valid rectangle (center tap goes first with
    start=True and covers the full tile).
Bias is added during the PSUM->SBUF copy (VectorE tensor_scalar), writing
bf16 (host converts the gathered output back to f32).

Schedule (from trace analysis of the 110.9us baseline):
  - mid-stream MMs pace at N/2.4GHz + ~5ns already (roofline); the fat is
    the entry ramp (first data MM at 11.1us) and the tail (5.3us after the
    last MM).
  - rings are FIFO at ~175GB/s with ~1.5-2.3us DGE first-packet latency and
    DMA instrs can only issue after engine boot (~6.5us).  Entry plan:
      sync:   w cot0 (ready ~9.7) -> bias -> w cot1 -> x2 full
      scalar: x0 rows in tapered pieces 10/8/16/14/8 (first ready ~9.7,
              then always ahead of the ~8rows/0.85us compute cadence)
              -> x1 full -> x3 full
    gpsimd (slow SWDGE queue) carries nothing.
  - PE prewarm: dummy matmuls on an UNINITIALIZED tile (garbage bf16 is
    harmless: output goes to a never-read PSUM bank) start right after the
    Tensor engine boots with no cross-engine dependency, so the HAM clock
    gate is warm (~3.4us later) right when the first data arrives, and the
    first data chunks run at most briefly at the cold 1.2GHz rate (still
    doing real work, strictly better than idling).
  - tail: the last (img,cot) block tapers its chunks to 8x5,6,2 rows with
    stores (0,14)(14,28)(28,44)(44,54)(54,56) on alternating rings, so the
    exit barrier's trailing DMA is a 2-row (28KB) transfer instead of the
    baseline's 4-row store behind a 20-row one.
"""

import numpy as np
import ml_dtypes

import concourse.mybir as mybir
import concourse.tile as tile
from concourse import bacc
from concourse.bass_utils import run_bass_kernel_spmd

N_CORES = 8
B, CIN, H, W = 32, 128, 56, 56
COUT, R, S = 256, 3, 3
BL = B // N_CORES          # images per core
NCOT = COUT // 128         # Cout tiles of 128
YCHUNK = 8                 # output rows per matmul tile
NYC = H // YCHUNK

MM_DT = mybir.dt.bfloat16
MM_NP = ml_dtypes.bfloat16

NWARM = 10                 # dummy matmuls bridging entry barrier -> first data
WARM_N = 128               # free dim of each dummy matmul (small => tiny memset)
X0_SPLITS = [(0, 10), (10, 18), (18, 34), (34, 48), (48, 56)]
OUT_SPLITS = {1: (0, 14), 3: (14, 28), 5: (28, 48), 6: (48, 56)}  # yc -> store rows
# tap order in the weight layout: center tap first (it is the start=True
# matmul that covers the full PSUM tile)
TAP_ORDER = [(1, 1), (0, 0), (0, 1), (0, 2), (1, 0), (1, 2), (2, 0), (2, 1), (2, 2)]

_cache = {}


def _build():
    if "nc" in _cache:
        return _cache["nc"]
    nc = bacc.Bacc("TRN2", target_bir_lowering=False, debug=False)
    f32 = mybir.dt.float32
    x_d = nc.dram_tensor("x", [BL, CIN, H, W], MM_DT, kind="ExternalInput").ap()
    w_d = nc.dram_tensor("w", [CIN, NCOT, R * S, 128], MM_DT, kind="ExternalInput").ap()
    b_d = nc.dram_tensor("b", [128, NCOT], f32, kind="ExternalInput").ap()
    y_d = nc.dram_tensor("y", [BL, COUT, H, W], MM_DT, kind="ExternalOutput").ap()

    with tile.TileContext(nc) as tc:
        with (
            tc.tile_pool(name="consts", bufs=1) as cpool,
            tc.tile_pool(name="xin", bufs=BL) as xpool,
            tc.tile_pool(name="yout", bufs=3) as opool,
            tc.tile_pool(name="ps", bufs=8, space="PSUM") as pspool,
        ):
            # --- PE prewarm: the warm tile is tiny so the VectorE memset it
            # depends on retires ~0.4us earlier than a full-width one ---
            warm_x = cpool.tile([128, WARM_N], MM_DT)
            nc.vector.memset(warm_x[:], 0.0)
            warm_ps = pspool.tile([128, WARM_N], f32, tag="ps")
            for _ in range(NWARM):
                nc.tensor.matmul(
                    warm_ps[:], warm_x[:, 0:128], warm_x[:], start=True, stop=True
                )

            # --- loads, in intended per-ring FIFO order ---
            w_sb = cpool.tile([CIN, NCOT, R * S, 128], MM_DT)
            b_sb = cpool.tile([128, NCOT], f32)
            x_tiles = [
                xpool.tile([CIN, H, W], MM_DT, name=f"x_sb_{img}", tag="x_sb")
                for img in range(BL)
            ]
            # sync ring: critical weights, bias, then bulk
            nc.sync.dma_start(w_sb[:, 0], w_d[:, 0])
            nc.sync.dma_start(b_sb[:], b_d[:])
            nc.sync.dma_start(w_sb[:, 1], w_d[:, 1])
            nc.sync.dma_start(x_tiles[2][:], x_d[2])
            # scalar ring: x0 prefix pieces sized to the compute cadence
            for r0, r1 in X0_SPLITS:
                nc.scalar.dma_start(x_tiles[0][:, r0:r1, :], x_d[0, :, r0:r1, :])
            nc.scalar.dma_start(x_tiles[1][:], x_d[1])
            nc.scalar.dma_start(x_tiles[3][:], x_d[3])

            store_rings = [nc.sync, nc.scalar]
            store_cnt = 0

            norm_chunks = [(YCHUNK * yc, YCHUNK) for yc in range(NYC)]
            norm_stores = dict(OUT_SPLITS)
            # last block: taper to a 2-row final chunk so the exit barrier
            # only waits on a tiny trailing transfer
            last_chunks = norm_chunks[:-1] + [(48, 6), (54, 2)]
            last_stores = {
                1: (0, 14), 3: (14, 28), 5: (28, 44), 6: (44, 54), 7: (54, 56)
            }

            for img in range(BL):
                x_sb = x_tiles[img]
                for cot in range(NCOT):
                    last_block = img == BL - 1 and cot == NCOT - 1
                    chunks = last_chunks if last_block else norm_chunks
                    stores = last_stores if last_block else norm_stores
                    o_sb = opool.tile(
                        [128, H, W], MM_DT, name=f"o_sb_{img}_{cot}", tag="o_sb"
                    )
                    for yc, (y0, rows) in enumerate(chunks):
                        ps = pspool.tile(
                            [128, rows, W], f32, name=f"ps_{img}_{cot}_{yc}", tag="ps"
                        )
                        # center tap first: full-tile write with start=True
                        nc.tensor.matmul(
                            ps[:],
                            w_sb[:, cot, 0, :],
                            x_sb[:, y0 : y0 + rows, :],
                            start=True,
                            stop=False,
                        )
                        for ti, (ky, kx) in enumerate(TAP_ORDER[1:], start=1):
                            oy0 = max(0, 1 - ky - y0)
                            oy1 = min(rows, H + 1 - y0 - ky)
                            ox0 = max(0, 1 - kx)
                            ox1 = min(W, W + 1 - kx)
                            nc.tensor.matmul(
                                ps[:, oy0:oy1, ox0:ox1],
                                w_sb[:, cot, ti, :],
                                x_sb[
                                    :,
                                    y0 + oy0 + ky - 1 : y0 + oy1 + ky - 1,
                                    ox0 + kx - 1 : ox1 + kx - 1,
                                ],
                                start=False,
                                stop=(ti == R * S - 1),
                            )
                        # PSUM -> SBUF with fused bias add, all on VectorE
                        # (no ACTIVATE => Scalar never loads its LUT)
                        nc.vector.tensor_scalar_add(
                            o_sb[:, y0 : y0 + rows],
                            ps[:],
                            b_sb[:, cot : cot + 1],
                        )
                        # store finished row bands, alternating fast rings
                        if yc in stores:
                            r0, r1 = stores[yc]
                            eng = store_rings[store_cnt % 2]
                            store_cnt += 1
                            eng.dma_start(
                                y_d[img, 128 * cot : 128 * (cot + 1), r0:r1, :],
                                o_sb[:, r0:r1, :],
                            )

    nc.compile()
    _cache["nc"] = nc
    return nc


def _in_maps(inputs, weight, bias):
    x = np.asarray(inputs).astype(MM_NP)
    # weight (co, ci, ky, kx) -> (ci, cot, tap, co_in_tile), taps in TAP_ORDER
    wt = (
        np.asarray(weight)
        .reshape(NCOT, 128, CIN, R, S)
        .transpose(2, 0, 3, 4, 1)  # (ci, cot, ky, kx, co)
        .astype(MM_NP)
    )
    w = np.ascontiguousarray(
        np.stack([wt[:, :, ky, kx, :] for ky, kx in TAP_ORDER], axis=2)
    )
    b = np.ascontiguousarray(
        np.asarray(bias).astype(np.float32).reshape(NCOT, 128).T
    )
    return [
        {"x": np.ascontiguousarray(x[c * BL : (c + 1) * BL]), "w": w, "b": b}
        for c in range(N_CORES)
    ]


def kernel(inputs, weight, bias):
    nc = _build()
    in_maps = _in_maps(inputs, weight, bias)
    res = run_bass_kernel_spmd(nc, in_maps, core_ids=list(range(N_CORES)))
    out = np.concatenate([res.results[c]["y"] for c in range(N_CORES)], axis=0)
    return out.astype(np.float32)
